# revision 25
# baseline (speedup 1.0000x reference)
"""MultiHeadAttention (B=2, S=4096, D=512, H=8) on 8 TRN2 NeuronCores.

Sharding: core c -> batch b = c//4, head-pair hp = c%4 (heads 2*hp, 2*hp+1).
Each core computes the partial output  concat(O_h0,O_h1) @ Wo[:,128cols].T
for its batch; host sums the 4 partials per batch and adds bo + Wo@bv.

Math notes (exact rewrites, not approximations):
  - K-bias bk drops out of softmax (adds a per-query constant to scores).
  - V-bias bv passes through softmax unchanged -> becomes the constant
    Wo@bv added on the host.
  - 1/sqrt(64) is folded into Wq and bq on the host.

Device dataflow per (head, 1024-query group, 128-key chunk):
  scoresT[k,q] = (K-chunk @ QT) in PSUM  (keys on partitions)
  probsT = exp(scoresT): ~2/3 of chunks on ScalarE (native Exp); a fixed
    3-in-8 subset on VectorE via the Schraudolph bit-trick (affine ->
    int16 convert whose bits are the bf16 pattern of 2^(s*log2e);
    rel err ~1.8% rms on those chunks, well under the 2e-2 budget).
  O[q,hd] accumulated in PSUM via 8 matmuls per chunk with
    lhsT=probsT[:,128q] (M=128 queries), rhs=V_chunk (N=64): full
    PE-array utilization, half the cycles of the O^T orientation.
    start=True only on the first sub-accumulator of the bank (start
    arms the whole 2KB bank as pending-zero; the other sub-accs'
    first writes auto-replace).
  softmax denominators ride a persistent always-accumulate PSUM bank
    (pre-zeroed once; never armed with start) fed by N=1 ones-matmuls,
    so they land on q partitions where 1/sums is a legal scalar.
  normalization happens at the PSUM drain (VectorE),
  O^T for the output projection is built by XBAR DMA-transpose,
  output projection is a single K=128 matmul per 128-query block.

Pipelining: scores are emitted TWO chunks ahead of attn@V so exp(i+2)
starts the moment exp(i) frees its double-buffered score PSUM slot;
K/V projection blocks are spread through (g0,h0) paced to the x^T DMA
stream; output projections for group g spread through (g+1,h0).
"""

import os
import sys

sys.path.insert(0, "/opt/trn_rl_repo")

import numpy as np
import ml_dtypes

import concourse.bass as bass
import concourse.bacc as bacc
import concourse.tile as tile
import concourse.mybir as mybir
import concourse.bass_utils as bass_utils

BF16 = ml_dtypes.bfloat16
F32 = np.float32
DT = mybir.dt
ALU = mybir.AluOpType

S = 4096
D = 512
HD = 64
NCORES = 8

# Schraudolph-style exp2 constants for the VectorE bit-trick chunks:
# bits_bf16(exp(s)) ~= int16(s * 128*log2(e) + (127 + C)*128 + 0.5)
_EXP_C = -0.058
EXP_A = float(128.0 * np.log2(np.e))
EXP_B = float((127.0 + _EXP_C) * 128.0 + 0.5)

_CACHE = {}


def _build_module():
    nc = bacc.Bacc(
        "TRN2",
        target_bir_lowering=False,
        debug=False,
        enable_asserts=False,
        num_devices=NCORES,
    )
    xT_d = nc.dram_tensor("xT", (D, S), DT.bfloat16, kind="ExternalInput").ap()
    # packed [Wq^T | Wk^T | Wv^T | bq(bf16) pad] so one DMA loads every
    # projection weight (HWDGE overhead is per-instruction)
    wpack_d = nc.dram_tensor("wpack", (D, 386), DT.bfloat16, kind="ExternalInput").ap()
    woT_d = nc.dram_tensor("woT", (128, D), DT.bfloat16, kind="ExternalInput").ap()
    out_d = nc.dram_tensor("out", (S, D), DT.bfloat16, kind="ExternalOutput").ap()

    with tile.TileContext(nc) as tc:
        with (
            tc.tile_pool(name="const", bufs=1) as cpool,
            tc.tile_pool(name="probs", bufs=14) as ppool,
            tc.tile_pool(name="rec", bufs=6) as rpool,
            tc.tile_pool(name="osb", bufs=8) as opool,
            tc.tile_pool(name="psum", bufs=2, space="PSUM") as psum,
        ):
            # ---- constants ----
            wqkv = cpool.tile([128, 4, 386], DT.bfloat16)
            wq = wqkv[:, :, 0:128]
            wk = wqkv[:, :, 128:256]
            wv = wqkv[:, :, 256:384]
            bqs_bf = wqkv[:, 0, 384:385]  # bf16 bias column
            xt = cpool.tile([128, 4, S], DT.bfloat16)  # x^T, contraction-chunked
            wo = cpool.tile([128, D], DT.bfloat16)
            ones = cpool.tile([128, 1], DT.bfloat16)

            def xt_slice(sb):
                nc.sync.dma_start(
                    xt[:, :, sb * 512 : (sb + 1) * 512],
                    xT_d[:, sb * 512 : (sb + 1) * 512].rearrange(
                        "(c p) m -> p c m", p=128
                    ),
                )

            nc.sync.dma_start(wqkv[:], wpack_d.rearrange("(c p) m -> p c m", p=128))
            xt_slice(0)
            xt_slice(1)
            xt_slice(2)
            xt_slice(3)
            nc.sync.dma_start(wo[:], woT_d[:])
            for sb in range(4, 8):
                xt_slice(sb)

            # ---- persistent SBUF tensors ----
            qt = cpool.tile([128, S], DT.bfloat16)  # Q^T (2 heads stacked)
            kt = cpool.tile([128, S], DT.bfloat16)  # K^T
            vnat = cpool.tile([128, 32, 2, 64], DT.bfloat16)  # V, [k, (head, hd)]
            obuf = cpool.tile([128, 32, 2, 64], DT.bfloat16)  # normalized O
            obufB = cpool.tile([128, 4, 2, 64], DT.bfloat16)  # g3 odd sub-blocks
            otn = cpool.tile([128, 4, 8, 128], DT.bfloat16)  # O^T blocks

            nc.vector.memset(ones[:], 1.0)
            # scalar bias operands must be fp32: widen the packed bf16 bq once
            bqs = cpool.tile([128, 1], DT.float32)
            nc.vector.tensor_copy(bqs[:], bqs_bf)

            # persistent softmax-denominator bank: never armed by start=True
            # (arming is bank-granular and would clobber the other in-flight
            # generation); pre-zeroed once, each region written exactly once.
            sums_ps = psum.tile([128, 64], DT.float32, tag="sums", name="sums_ps", bufs=1)
            nc.vector.memset(sums_ps[:], 0.0)

            # ---- PE p-state warmup: ~3us of dependency-free dummy matmuls
            # so the Tensor engine reaches full clock while the first input
            # DMAs are still in flight ----
            wdum = cpool.tile([1, 640], DT.bfloat16)
            nc.vector.memset(wdum[:], 0.0)
            pdum = psum.tile([128, 128], DT.float32, tag="small", name="pdum", bufs=1)
            for _ in range(30):
                nc.tensor.matmul(
                    pdum[:], wdum[:, 0:128], wdum[:, 128:256],
                    start=True, stop=True, skip_group_check=True,
                )

            # ---- projections ----
            def proj_block(dst, w, bias, sb, tag="small"):
                pt = psum.tile([128, 512], DT.float32, tag=tag, name="pt",
                               bufs=1 if tag == "small" else None)
                for kc in range(4):
                    nc.tensor.matmul(
                        pt[:],
                        w[:, kc, :],
                        xt[:, kc, sb * 512 : (sb + 1) * 512],
                        start=(kc == 0),
                        stop=(kc == 3),
                    )
                if bias is not None:
                    nc.vector.tensor_scalar(
                        dst[:, sb * 512 : (sb + 1) * 512],
                        pt[:],
                        bias[:, 0:1],
                        None,
                        ALU.add,
                    )
                else:
                    nc.vector.tensor_copy(dst[:, sb * 512 : (sb + 1) * 512], pt[:])

            def v_block4(b, tag="small"):
                # V-natural for key chunks 4b..4b+3 in one [128,512] psum
                pt = psum.tile([128, 512], DT.float32, tag=tag, name="pt_v",
                               bufs=1 if tag == "small" else None)
                for j in range(4):
                    for kc in range(4):
                        nc.tensor.matmul(
                            pt[:, j * 128 : (j + 1) * 128],
                            xt[:, kc, (4 * b + j) * 128 : (4 * b + j + 1) * 128],
                            wv[:, kc, :],
                            start=(kc == 0),
                            stop=(kc == 3),
                            skip_group_check=True,
                        )
                nc.vector.tensor_copy(
                    vnat[:, 4 * b : 4 * b + 4, :, :],
                    pt[:].rearrange("p (j a b) -> p j a b", j=4, a=2),
                )

            # minimal prefix so the exp engines start almost immediately;
            # prefix projections ride the (still idle) score-PSUM ring
            proj_block(kt, wk, None, 0, tag="scL")
            proj_block(qt, wq, bqs, 0, tag="scL")

            # ---- attention, query-group-major, software-pipelined ----
            chunks = [(g, h, kc) for g in range(4) for h in range(2) for kc in range(32)]

            def exp_on_dve(i):
                import os as _os
                if _os.environ.get("NO_DVE_EXP"):
                    return False
                if i < 8:
                    return False
                return i % 12 in (1, 3, 6, 8, 10)

            acc_t = {}
            pb_t = {}
            pending_finals = {}

            def emit_scores_exp(i, halves=(0, 1)):
                g, h, kc = chunks[i]
                if g == 0 and h == 0:
                    # spread K-proj and V-natural blocks through (g0,h0),
                    # paced so each lands ~2 chunks before first use and
                    # after its x^T DMA slice has arrived
                    if kc % 4 == 1 and 1 <= (kc + 3) // 4 <= 7:
                        proj_block(kt, wk, None, (kc + 3) // 4)
                    if kc % 4 == 2 and 1 <= (kc + 2) // 4 <= 7:
                        v_block4((kc + 2) // 4)
                if h == 1 and g <= 2 and kc in (20, 26):
                    proj_block(qt, wq, bqs, 2 * (g + 1) + (kc == 26))
                hr = h * 64
                q0 = g * 1024
                # two independent [128,512] score halves: exp(i+2)'s half can
                # start as soon as exp(i)'s matching half frees its slot, and
                # attn@V's first 4 sub-blocks only wait for the left half
                if halves == (0, 1) or halves == (0,):
                    pb = ppool.tile([128, 1024], DT.bfloat16, name="pb")
                    pb_t[(g, h, kc)] = pb
                else:
                    pb = pb_t[(g, h, kc)]
                dve = exp_on_dve(i)
                for qh, tag in [((0, "scL"), (1, "scR"))[q] for q in halves]:
                    sch = psum.tile([128, 512], DT.float32, tag=tag, name=tag)
                    nc.tensor.matmul(
                        sch[:],
                        kt[hr : hr + 64, kc * 128 : (kc + 1) * 128],
                        qt[hr : hr + 64, q0 + qh * 512 : q0 + (qh + 1) * 512],
                        start=True,
                        stop=True,
                    )
                    half = pb[:, qh * 512 : (qh + 1) * 512]
                    if dve:
                        nc.vector.tensor_scalar(
                            half.bitcast(DT.int16), sch[:], EXP_A, EXP_B,
                            ALU.mult, ALU.add,
                        )
                    else:
                        nc.scalar.activation(
                            half, sch[:], mybir.ActivationFunctionType.Exp
                        )

            def emit_av(i):
                g, h, kc = chunks[i]
                pb = pb_t.pop((g, h, kc))
                last = g == 3 and h == 1
                if kc == 0:
                    if last:
                        # the final generation drains on the critical path:
                        # split it across two tiles (one per drain engine)
                        acc_t[(g, h)] = (
                            psum.tile([128, 256], DT.float32, tag="acc", name="accA"),
                            psum.tile([128, 256], DT.float32, tag="acc", name="accB"),
                        )
                    else:
                        acc_t[(g, h)] = (
                            psum.tile([128, 512], DT.float32, tag="acc", name="acc"),
                        )
                acc = acc_t[(g, h)]
                col0 = (g * 2 + h) * 8
                for qc in range(8):
                    # start=True arms the whole 2KB PSUM bank as pending-zero,
                    # so only the first sub-accumulator per bank may set it;
                    # the other sub-accs' first writes land on armed bytes and
                    # auto-replace.
                    if last:
                        dst = acc[qc // 4][:, (qc % 4) * 64 : (qc % 4 + 1) * 64]
                        st = kc == 0 and qc % 4 == 0
                    else:
                        dst = acc[0][:, qc * 64 : (qc + 1) * 64]
                        st = kc == 0 and qc == 0
                    nc.tensor.matmul(
                        dst,
                        pb[:, qc * 128 : (qc + 1) * 128],
                        vnat[:, kc, h, :],
                        start=st,
                        stop=(kc == 31),
                        skip_group_check=True,
                    )
                for qc in range(8):
                    nc.tensor.matmul(
                        sums_ps[:, col0 + qc : col0 + qc + 1],
                        pb[:, qc * 128 : (qc + 1) * 128],
                        ones[:],
                        start=False,
                        stop=(kc == 31),
                        skip_group_check=True,
                    )
                if kc == 31:
                    emit_drain(g, h)
                # during g3, hold back half of g2's output blocks: they are
                # emitted after the last attn@V chunk to keep the PE p-state
                # warm across the final drain/transpose latency gap
                pop_now = kc % 4 == 3 if g < 3 else kc % 8 == 3
                if h == 0 and pop_now and pending_finals.get(g - 1):
                    emit_final_block(*pending_finals[g - 1].pop(0))

            def emit_drain(g, h):
                acc = acc_t.pop((g, h))
                col0 = (g * 2 + h) * 8
                last = g == 3 and h == 1
                # drains stay per-tile-single-engine: cross-engine access to
                # shared tiles serializes on per-tile semaphores. The final
                # generation splits acc/recs/obuf per engine so its drain,
                # which sits on the critical path, runs on both engines in
                # parallel. g3 uses a remapped obuf layout (even q-blocks in
                # obuf[24:28], odd in obufB) so each transpose input stays
                # contiguous.
                if last:
                    recsA = rpool.tile([128, 4], DT.float32, name="recsA")
                    recsB = rpool.tile([128, 4], DT.float32, name="recsB")
                    nc.vector.reciprocal_approx_fast(
                        recsA[:], sums_ps[:, col0 : col0 + 4]
                    )
                    nc.vector.reciprocal_approx_fast(
                        recsB[:], sums_ps[:, col0 + 4 : col0 + 8]
                    )
                    for qc in range(8):
                        src_ap = acc[qc // 4][:, (qc % 4) * 64 : (qc % 4 + 1) * 64]
                        dsto = obuf[:, 24 + qc // 2, 1, :] if qc % 2 == 0 else (
                            obufB[:, qc // 2, 1, :]
                        )
                        if qc < 4:
                            nc.vector.tensor_scalar(
                                dsto, src_ap, recsA[:, qc : qc + 1], None, ALU.mult
                            )
                        else:
                            nc.scalar.activation(
                                dsto, src_ap,
                                mybir.ActivationFunctionType.Copy,
                                scale=recsB[:, qc - 4 : qc - 3],
                            )
                else:
                    recs = rpool.tile([128, 8], DT.float32, name="recs")
                    nc.vector.reciprocal_approx_fast(
                        recs[:], sums_ps[:, col0 : col0 + 8]
                    )
                    for qc in range(8):
                        if g < 3:
                            dsto = obuf[:, g * 8 + qc, h, :]
                        elif qc % 2 == 0:
                            dsto = obuf[:, 24 + qc // 2, h, :]
                        else:
                            dsto = obufB[:, qc // 2, h, :]
                        nc.vector.tensor_scalar(
                            dsto,
                            acc[0][:, qc * 64 : (qc + 1) * 64],
                            recs[:, qc : qc + 1], None, ALU.mult,
                        )
                if h == 1:
                    if g < 3:
                        nc.sync.dma_start_transpose(
                            otn[:, g, :, :], obuf[:, g * 8 : (g + 1) * 8, :, :]
                        )
                    else:
                        nc.sync.dma_start_transpose(
                            otn[:, 3, 0:8:2, :], obuf[:, 24:28, :, :]
                        )
                        nc.scalar.dma_start_transpose(
                            otn[:, 3, 1:8:2, :], obufB[:, :, :, :]
                        )
                    pending_finals[g] = [(g, i) for i in range(8)]

            def emit_final_block(g, i):
                r0 = (g * 8 + i) * 128
                # tail finals rotate through both (now idle) score rings for
                # 4-deep PSUM pipelining; mid-run finals use the small bank
                tag = ("scL", "scR")[i % 2] if g == 3 else "small"
                po = psum.tile([128, 512], DT.float32, tag=tag, name="po",
                               bufs=1 if tag == "small" else None)
                nc.tensor.matmul(po[:], otn[:, g, i, :], wo[:], start=True, stop=True)
                o = opool.tile([128, 512], DT.bfloat16, name="o")
                if g == 3 and i % 2 == 1:
                    nc.scalar.activation(
                        o[:], po[:], mybir.ActivationFunctionType.Copy
                    )
                else:
                    nc.vector.tensor_copy(o[:], po[:])
                eng = (nc.sync, nc.scalar)[i % 2] if g == 3 else nc.sync
                eng.dma_start(out_d[r0 : r0 + 128, :], o[:])

            # scores run TWO chunks ahead of attn@V: with double-buffered
            # score PSUM, exp(i+2)'s input is ready the moment exp(i) frees
            # its slot, so consecutive same-engine exps pipeline instead of
            # serializing through attn@V -> scores -> exp. The left halves of
            # the first two chunks only need qt columns 0:512, so they start
            # before the second Q-projection block.
            emit_scores_exp(0, halves=(0,))
            emit_scores_exp(1, halves=(0,))
            proj_block(qt, wq, bqs, 1, tag="scR")
            v_block4(0)
            emit_scores_exp(0, halves=(1,))
            emit_scores_exp(1, halves=(1,))
            for i in range(len(chunks)):
                if i + 2 < len(chunks):
                    emit_scores_exp(i + 2)
                emit_av(i)
            for g, i in pending_finals.get(2, []):
                emit_final_block(g, i)
            for g, i in pending_finals.get(3, []):
                emit_final_block(g, i)

    nc.compile()
    return nc


def _get_module():
    if "nc" not in _CACHE:
        _CACHE["nc"] = _build_module()
    return _CACHE["nc"]


def _prep_in_maps(x, Wq, bq, Wk, bk, Wv, bv, Wo, bo):
    in_maps = []
    wqT = np.ascontiguousarray((Wq / 8.0).T.astype(BF16))
    wkT = np.ascontiguousarray(Wk.T.astype(BF16))
    wvT = np.ascontiguousarray(Wv.T.astype(BF16))
    woT = np.ascontiguousarray(Wo.T.astype(BF16))
    bq8 = (bq / 8.0).astype(BF16)
    xTb = [np.ascontiguousarray(x[b].T.astype(BF16)) for b in range(2)]
    for c in range(NCORES):
        b = c // 4
        js = slice((c % 4) * 128, (c % 4 + 1) * 128)
        wpack = np.zeros((D, 386), dtype=BF16)
        wpack[:, 0:128] = wqT[:, js]
        wpack[:, 128:256] = wkT[:, js]
        wpack[:, 256:384] = wvT[:, js]
        wpack[0:128, 384] = bq8[js]
        in_maps.append(
            {
                "xT": xTb[b],
                "wpack": wpack,
                "woT": np.ascontiguousarray(woT[js, :]),
            }
        )
    return in_maps


def kernel(x, Wq, bq, Wk, bk, Wv, bv, Wo, bo, _trace=False):
    x = np.asarray(x, dtype=np.float32)
    nc = _get_module()
    in_maps = _prep_in_maps(
        x, np.asarray(Wq), np.asarray(bq), np.asarray(Wk), np.asarray(bk),
        np.asarray(Wv), np.asarray(bv), np.asarray(Wo), np.asarray(bo),
    )
    res = None
    for attempt in range(3):
        try:
            res = bass_utils.run_bass_kernel_spmd(
                nc, in_maps, core_ids=list(range(NCORES)), trace=_trace
            )
            break
        except Exception:
            # transient NRT device wedge: retry with a freshly rebuilt module
            if attempt == 2:
                raise
            _CACHE.clear()
            nc = _get_module()
    const = (np.asarray(bo) + np.asarray(Wo) @ np.asarray(bv)).astype(F32)
    out = np.empty((2, S, D), dtype=np.float32)
    for b in range(2):
        acc = res.results[4 * b]["out"].astype(np.float64)
        for c in range(4 * b + 1, 4 * b + 4):
            acc = acc + res.results[c]["out"]
        out[b] = (acc + const).astype(np.float32)
    if _trace:
        return out, res
    return out
